# revision 16
# baseline (speedup 1.0000x reference)
"""Multi-head self-attention (B=2, T=2048, D=1024, 16 heads) on 8 TRN2 cores.

Sharding: core c = (b, g) with b = c // 4 (batch), g = c % 4 (head group of 4).
Each core computes q/k/v projections for its 4 heads, causal softmax
attention, and a partial output projection (its 256 columns of the
concat-head dim against Wo). Host sums the 4 partials per batch and adds bo.

v2: single fully-interleaved pass. The per-(head-pair, query-block)
attention pipeline (scoresT matmul -> ACT exp -> AV accumulate) is the
backbone; projection groups and output-projection token-chunks are
injected as PE "fillers" between attention chunks so the tensor engine
never idles while the scalar engine (exp) runs, and vice versa. exp is
the only ACT table function used (plus Identity for bias-folded copies,
same table set) -- zero ACT table reloads. Softmax denominators are
inverted with the custom-DVE reciprocal_approx_fast (~18 bits) and
applied via gpsimd partition-broadcast + DVE multiply.

Per-core layout:
  qT/kT [128, 2048] bf16: rows = 2 heads x 64 dims, cols = tokens.
  vt[t] [128, 512] bf16: rows = 128 key tokens of chunk t, cols =
     4 heads x 128 (per head: 64 v dims, a 1.0 column at 64 that makes
     the AV matmul emit softmax denominators, zero padding).
  attT [128, 2048] bf16 per head pair: normalized A^T V rows.
  O [2048, 1024] bf16 partial output, summed on host in f32.
"""

import ml_dtypes
import numpy as np

import concourse.bass as bass
import concourse.tile as tile
from concourse import bacc, mybir
from concourse import bass_utils
from contextlib import ExitStack

F32 = mybir.dt.float32
BF16 = mybir.dt.bfloat16
ATT = BF16
AF = mybir.ActivationFunctionType
OP = mybir.AluOpType

import os
DEBUG_DUMP = os.environ.get("BASS_DEBUG_DUMP", "0") == "1"

B, T, D = 2, 2048, 1024
NH, DH = 16, 64
HPC = 4            # heads per core
GD = HPC * DH      # 256, group dim
GV = HPC * (DH + 1)  # 260, v tile width (compact, 65 per head)
NKD = D // 128     # 8 K-chunks for projections
NT = T // 128      # 16 token chunks
NJ = T // 512      # 4 query blocks

_NC_CACHE = {}


def build():
    if "nc" in _NC_CACHE:
        return _NC_CACHE["nc"]
    nc = bacc.Bacc("TRN2", target_bir_lowering=False, debug=False, num_devices=8)

    HT = nc.dram_tensor("HT", [D, T], BF16, kind="ExternalInput").ap()
    WqT = nc.dram_tensor("WqT", [D, GD], BF16, kind="ExternalInput").ap()
    WkT = nc.dram_tensor("WkT", [D, GD], BF16, kind="ExternalInput").ap()
    WvS = nc.dram_tensor("WvS", [D, GV], BF16, kind="ExternalInput").ap()
    WoS = nc.dram_tensor("WoS", [GD, D], BF16, kind="ExternalInput").ap()
    bqc = nc.dram_tensor("bqc", [128, 2], F32, kind="ExternalInput").ap()
    bkc = nc.dram_tensor("bkc", [128, 2], F32, kind="ExternalInput").ap()
    bvS = nc.dram_tensor("bvS", [1, GV], F32, kind="ExternalInput").ap()
    kpm = nc.dram_tensor("kpm", [128, NT], F32, kind="ExternalInput").ap()
    O = nc.dram_tensor("O", [T, D], BF16, kind="ExternalOutput").ap()
    zd = nc.dram_tensor("zd", [8, 1024], F32, kind="Internal").ap()
    rd = nc.dram_tensor("rd", [8, 1024], F32, kind="Internal").ap()
    if DEBUG_DUMP:
        dbg = {
            "qT_d": [nc.dram_tensor(f"qT_d{m}", [128, T], BF16, kind="ExternalOutput").ap() for m in range(2)],
            "kT_d": [nc.dram_tensor(f"kT_d{m}", [128, T], BF16, kind="ExternalOutput").ap() for m in range(2)],
            "vt_d": nc.dram_tensor("vt_d", [NT * 128, 512], BF16, kind="ExternalOutput").ap(),
            "attT_d": [nc.dram_tensor(f"attT_d{m}", [128, T], BF16, kind="ExternalOutput").ap() for m in range(2)],
            "z_d": nc.dram_tensor("z_d", [1, 16 * 512], F32, kind="ExternalOutput").ap(),
            "rinv_d": nc.dram_tensor("rinv_d", [1, 16 * 512], F32, kind="ExternalOutput").ap(),
            "rb_d": nc.dram_tensor("rb_d", [64, 512], F32, kind="ExternalOutput").ap(),
        }

    with tile.TileContext(nc) as tc, ExitStack() as octx:
        cpool = octx.enter_context(tc.tile_pool(name="const", bufs=1))
        keep = octx.enter_context(tc.tile_pool(name="keep", bufs=1))
        work = octx.enter_context(tc.tile_pool(name="work", bufs=1))
        ps_sc = octx.enter_context(tc.tile_pool(name="ps_sc", bufs=1, space="PSUM"))
        ps_at = octx.enter_context(tc.tile_pool(name="ps_at", bufs=1, space="PSUM"))
        ps_sm = octx.enter_context(tc.tile_pool(name="ps_sm", bufs=1, space="PSUM"))

        # ---- constants ----
        bq_sb = cpool.tile([128, 2], F32, name="bq_sb", tag="bq_sb")
        bk_sb = cpool.tile([128, 2], F32, name="bk_sb", tag="bk_sb")
        bv_sb = cpool.tile([1, GV], F32, name="bv_sb", tag="bv_sb")
        kpm_sb = cpool.tile([128, NT], F32, name="kpm_sb", tag="kpm_sb")
        nc.sync.dma_start(bq_sb[:], bqc[:])
        nc.sync.dma_start(bk_sb[:], bkc[:])
        nc.sync.dma_start(bv_sb[:], bvS[:])
        nc.sync.dma_start(kpm_sb[:], kpm[:])

        # bv broadcast across partitions: [128, GV]
        bvrow = cpool.tile([128, GV], F32, name="bvrow", tag="bvrow")
        nc.gpsimd.partition_broadcast(bvrow[:], bv_sb[:])

        # lower-tri mask [128,128]: keep where f >= p
        tri = cpool.tile([128, 128], ATT, name="tri", tag="tri")
        nc.gpsimd.memset(tri[:], 1.0)
        nc.gpsimd.affine_select(
            out=tri[:], in_=tri[:], compare_op=OP.is_ge, fill=0.0,
            base=0, pattern=[[1, 128]], channel_multiplier=-1,
        )

        # ---- long-lived activations ----
        qT = [keep.tile([128, T], ATT, name=f"qT{m}", tag=f"qT{m}") for m in range(2)]
        kT = [keep.tile([128, T], ATT, name=f"kT{m}", tag=f"kT{m}") for m in range(2)]
        vt = [keep.tile([128, 512], ATT, name=f"vt{t}", tag=f"vt{t}") for t in range(NT)]
        attT = [keep.tile([128, T], ATT, name=f"attT{m}", tag=f"attT{m}") for m in range(2)]
        wo_r = [keep.tile([128, D], ATT, name=f"wo{i}", tag=f"wo{i}") for i in range(2)]

        # zero vt padding once (only cols [h*128+65 : (h+1)*128) stay zero)
        for t in range(NT):
            nc.gpsimd.memset(vt[t][:], 0.0)

        # ---- input DMA: k-chunk-ordered so projections can start early ----
        ht_r = [work.tile([128, T], BF16, name=f"ht{k}", tag=f"ht{k}") for k in range(NKD)]
        wq_r = work.tile([128, NKD * GD], BF16, name="wq_r", tag="wq_r")
        wk_r = work.tile([128, NKD * GD], BF16, name="wk_r", tag="wk_r")
        wv_r = work.tile([128, NKD * GV], BF16, name="wv_r", tag="wv_r")
        nc.sync.dma_start(
            wq_r[:].rearrange("p (k g) -> p k g", k=NKD),
            WqT[:].rearrange("(k p) g -> p k g", k=NKD),
        )
        nc.sync.dma_start(
            wk_r[:].rearrange("p (k g) -> p k g", k=NKD),
            WkT[:].rearrange("(k p) g -> p k g", k=NKD),
        )
        for k in range(NKD):
            nc.sync.dma_start(ht_r[k][:], HT[k * 128:(k + 1) * 128, :])
        nc.sync.dma_start(
            wv_r[:].rearrange("p (k g) -> p k g", k=NKD),
            WvS[:].rearrange("(k p) g -> p k g", k=NKD),
        )
        for i in range(2):
            nc.gpsimd.dma_start(wo_r[i][:], WoS[i * 128:(i + 1) * 128, :])

        # ---- filler work units (PE work injected between attention chunks) ----
        def proj_qk(w_r, dest, bias_sb, m, n):
            # dest[m][:, n*512:(n+1)*512] = sum_k W_k[:, m-block].T @ ht_k + bias
            ps = ps_sm.tile([128, 512], F32, name="pp", tag="smps", bufs=2)
            for k in range(NKD):
                nc.tensor.matmul(
                    ps[:],
                    w_r[:, k * GD + m * 128: k * GD + m * 128 + 128],
                    ht_r[k][:, n * 512:(n + 1) * 512],
                    start=(k == 0), stop=(k == NKD - 1),
                )
            # bias add folded into the PSUM->SBUF copy (ACT Identity, set 0)
            nc.scalar.activation(
                dest[m][:, n * 512:(n + 1) * 512], ps[:],
                AF.Identity, bias=bias_sb[:, m:m + 1],
            )

        def proj_v(t):
            # vt[t] strided per-head blocks = (sum_k ht_k_t.T @ WvS_k + bv) * kpm
            vp = ps_sm.tile([128, 512], F32, name="vp", tag="smps", bufs=2)
            for k in range(NKD):
                nc.tensor.matmul(
                    vp[:, 0:GV],
                    ht_r[k][:, t * 128:(t + 1) * 128],
                    wv_r[:, k * GV:(k + 1) * GV],
                    start=(k == 0), stop=(k == NKD - 1),
                )
            nc.vector.tensor_tensor(
                vt[t][:].rearrange("p (h c) -> p h c", c=128)[:, :, 0:65],
                vp[:, 0:GV].rearrange("p (h c) -> p h c", c=65),
                bvrow[:].rearrange("p (h c) -> p h c", c=65),
                op=OP.add,
            )
            nc.vector.tensor_scalar_mul(vt[t][:], vt[t][:], kpm_sb[:, t:t + 1])

        _ot_live = {}

        def out_half(t, n):
            # O[t-chunk, n-half] = sum_hp attT[hp][:, t-chunk].T @ WoS[hp][:, n-half]
            if n == 0:
                _ot_live[t] = work.tile([128, D], BF16, name="ot", tag="ot", bufs=3)
            ot = _ot_live[t]
            op = ps_sm.tile([128, 512], F32, name="op", tag="smps", bufs=2)
            for hp in range(2):
                nc.tensor.matmul(
                    op[:],
                    attT[hp][:, t * 128:(t + 1) * 128],
                    wo_r[hp][:, n * 512:(n + 1) * 512],
                    start=(hp == 0), stop=(hp == 1),
                )
            if n == 0:
                nc.vector.tensor_copy(ot[:, 0:512], op[:])
            else:
                nc.scalar.copy(ot[:, 512:1024], op[:])
                nc.sync.dma_start(O[t * 128:(t + 1) * 128, :], ot[:])
                del _ot_live[t]

        fillers = []

        def pop_filler():
            if fillers:
                fillers.pop(0)()

        # ---- attention block machinery ----
        if DEBUG_DUMP:
            zcoll = cpool.tile([1, 16 * 512], F32, name="zcoll", tag="zcoll")
            rcoll = cpool.tile([1, 16 * 512], F32, name="rcoll", tag="rcoll")

        def normalize(hp, J, at):
            # zau: unnormalized A^T V rows (0:64) + denominator row (64)
            zaus = []
            for hh in range(2):
                zau = work.tile([65, 512], F32, name="zau", tag="zau", bufs=4)
                nc.vector.tensor_copy(zau[:], at[hh][0:65, :])
                zaus.append(zau)
            if DEBUG_DUMP:
                bi = hp * 4 + J
                for hh in range(2):
                    r = 2 * bi + hh
                    nc.vector.tensor_copy(
                        zcoll[0:1, r * 512:(r + 1) * 512], zaus[hh][64:65, :]
                    )
            bi = hp * 4 + J
            # exact reciprocal on a partition-packed [128, 8] tile: bounce the
            # two denominator rows through DRAM (engines cannot cross
            # partitions; DMA can). 8 elem/lane keeps the iterative divide
            # at ~130 ns instead of 4.3 us on a [1, 512] row.
            for hh in range(2):
                nc.gpsimd.dma_start(zd[bi:bi + 1, hh * 512:(hh + 1) * 512],
                                    zaus[hh][64:65, :])
            zp = work.tile([128, 8], F32, name="zp", tag="zp", bufs=2)
            nc.gpsimd.dma_start(
                zp[:], zd[bi:bi + 1, :].rearrange("p (a b) -> (p a) b", b=8)
            )
            rp = work.tile([128, 8], F32, name="rp", tag="rp", bufs=2)
            nc.vector.reciprocal(rp[:], zp[:])
            nc.gpsimd.dma_start(
                rd[bi:bi + 1, :].rearrange("p (a b) -> (p a) b", b=8), rp[:]
            )
            for hh in range(2):
                zau = zaus[hh]
                rinv = work.tile([1, 512], F32, name="rinv", tag="rinv", bufs=4)
                nc.gpsimd.dma_start(rinv[:], rd[bi:bi + 1, hh * 512:(hh + 1) * 512])
                rb = work.tile([64, 512], F32, name="rb", tag="rb", bufs=4)
                nc.gpsimd.partition_broadcast(rb[:], rinv[:])
                if DEBUG_DUMP:
                    ri = 2 * bi + hh
                    nc.vector.tensor_copy(rcoll[0:1, ri * 512:(ri + 1) * 512], rinv[:])
                    if hp == 0 and J == 0 and hh == 0:
                        nc.sync.dma_start(dbg["rb_d"][:], rb[:])
                nc.vector.tensor_tensor(
                    attT[hp][hh * 64:(hh + 1) * 64, J * 512:(J + 1) * 512],
                    zau[0:64, :],
                    rb[:],
                    op=OP.mult,
                )

        def block(J, hp):
            at = [
                ps_at.tile([128, 512], F32, name=f"at{hh}", tag="av", bufs=2)
                for hh in range(2)
            ]
            # diagonal chunk first (full width, opens PSUM accumulation),
            # then off-diagonals, then narrow diagonals.
            kcs = [4 * J] + list(range(4 * J)) + [4 * J + i for i in range(1, 4)]

            def issue_sc_exp(kc):
                off = max(0, 128 * (kc - 4 * J))
                w = 512 - off
                sc = ps_sc.tile([128, 1024], F32, name="sc", tag="sc", bufs=2)
                for hh in range(2):
                    nc.tensor.matmul(
                        sc[:, hh * 512:hh * 512 + w],
                        kT[hp][hh * 64:(hh + 1) * 64, kc * 128:(kc + 1) * 128],
                        qT[hp][hh * 64:(hh + 1) * 64, J * 512 + off:(J + 1) * 512],
                        start=True, stop=True,
                        tile_position=(hh * 64, 0),
                    )
                ex = work.tile([128, 1024], ATT, name="ex", tag="ex", bufs=8)
                nc.scalar.activation(
                    ex[:].rearrange("p (h c) -> p h c", c=512)[:, :, 0:w],
                    sc[:].rearrange("p (h c) -> p h c", c=512)[:, :, 0:w],
                    AF.Exp, scale=0.125,
                )
                if off or kc == 4 * J:
                    for hh in range(2):
                        nc.vector.tensor_tensor(
                            ex[:, hh * 512:hh * 512 + 128],
                            ex[:, hh * 512:hh * 512 + 128],
                            tri[:],
                            op=OP.mult,
                        )
                return ex

            def issue_av(kc, ex, first, last):
                off = max(0, 128 * (kc - 4 * J))
                w = 512 - off
                for hh in range(2):
                    h = 2 * hp + hh
                    nc.tensor.matmul(
                        at[hh][:, off:512],
                        vt[kc][:, h * 128:(h + 1) * 128],
                        ex[:, hh * 512:hh * 512 + w],
                        start=first, stop=last,
                    )

            prev = None
            for ti, kc in enumerate(kcs):
                ex = issue_sc_exp(kc)
                pop_filler()
                if prev is not None:
                    issue_av(prev[0], prev[1], first=(prev[2] == 0), last=False)
                prev = (kc, ex, ti)
            issue_av(prev[0], prev[1], first=(prev[2] == 0), last=True)
            normalize(hp, J, at)

        # ---- bootstrap projections for block (J=0, hp=0) ----
        proj_qk(wq_r, qT, bq_sb, 0, 0)
        proj_qk(wk_r, kT, bk_sb, 0, 0)
        for t in range(4):
            proj_v(t)

        # remaining projection work, dependency-ordered
        for n in range(1, 4):
            fillers.append(lambda n=n: proj_qk(wq_r, qT, bq_sb, 0, n))
            fillers.append(lambda n=n: proj_qk(wk_r, kT, bk_sb, 0, n))
            for t in range(4 * n, 4 * n + 4):
                fillers.append(lambda t=t: proj_v(t))
        for n in reversed(range(4)):
            fillers.append(lambda n=n: proj_qk(wq_r, qT, bq_sb, 1, n))
            fillers.append(lambda n=n: proj_qk(wk_r, kT, bk_sb, 1, n))

        # ---- main pass ----
        # hp1 runs J descending: the big J=3 block comes first (making its
        # output-projection fillers available early) and the small J=0 block
        # lands last, shortening the final normalize->out tail.
        for hp, Js in ((0, range(NJ)), (1, reversed(range(NJ)))):
            for J in Js:
                block(J, hp)
                if hp == 1:
                    # attT for both head pairs at J is now final
                    for t in range(4 * J, 4 * J + 4):
                        for n in range(2):
                            fillers.append(lambda t=t, n=n: out_half(t, n))
        while fillers:
            fillers.pop(0)()

        if DEBUG_DUMP:
            for m in range(2):
                nc.sync.dma_start(dbg["qT_d"][m][:], qT[m][:])
                nc.sync.dma_start(dbg["kT_d"][m][:], kT[m][:])
                nc.sync.dma_start(dbg["attT_d"][m][:], attT[m][:])
            for t in range(NT):
                nc.sync.dma_start(dbg["vt_d"][t * 128:(t + 1) * 128, :], vt[t][:])
            nc.sync.dma_start(dbg["z_d"][:], zcoll[:])
            nc.sync.dma_start(dbg["rinv_d"][:], rcoll[:])

    nc.compile()
    _NC_CACHE["nc"] = nc
    return nc


def _prep_core_inputs(H, key_padding_mask, Wq, bq, Wk, bk, Wv, bv, Wo, bo):
    keep = 1.0 - np.asarray(key_padding_mask, dtype=np.float32)  # [B, T]
    bf = ml_dtypes.bfloat16
    in_maps = []
    for c in range(8):
        b, g = divmod(c, 4)
        sl = slice(g * GD, (g + 1) * GD)
        WvT = Wv[sl].T  # [D, GD]
        WvS = np.zeros((D, GV), dtype=np.float32)
        bvS = np.zeros((1, GV), dtype=np.float32)
        for h in range(HPC):
            WvS[:, h * 65:h * 65 + 64] = WvT[:, h * 64:(h + 1) * 64]
            bvS[0, h * 65:h * 65 + 64] = bv[sl][h * 64:(h + 1) * 64]
            bvS[0, h * 65 + 64] = 1.0
        in_maps.append({
            "HT": np.ascontiguousarray(H[b].T).astype(bf),
            "WqT": np.ascontiguousarray(Wq[sl].T).astype(bf),
            "WkT": np.ascontiguousarray(Wk[sl].T).astype(bf),
            "WvS": WvS.astype(bf),
            "WoS": np.ascontiguousarray(Wo[:, sl].T).astype(bf),
            "bqc": np.ascontiguousarray(bq[sl].reshape(2, 128).T.astype(np.float32)),
            "bkc": np.ascontiguousarray(bk[sl].reshape(2, 128).T.astype(np.float32)),
            "bvS": bvS,
            "kpm": np.ascontiguousarray(keep[b].reshape(NT, 128).T),
        })
    return in_maps


def kernel(H, key_padding_mask, Wq, bq, Wk, bk, Wv, bv, Wo, bo, _run_kwargs=None):
    H = np.asarray(H, dtype=np.float32)
    Wq = np.asarray(Wq, dtype=np.float32)
    Wk = np.asarray(Wk, dtype=np.float32)
    Wv = np.asarray(Wv, dtype=np.float32)
    Wo = np.asarray(Wo, dtype=np.float32)
    bq = np.asarray(bq, dtype=np.float32)
    bk = np.asarray(bk, dtype=np.float32)
    bv = np.asarray(bv, dtype=np.float32)
    bo = np.asarray(bo, dtype=np.float32)

    nc = build()
    in_maps = _prep_core_inputs(H, key_padding_mask, Wq, bq, Wk, bk, Wv, bv, Wo, bo)
    res = bass_utils.run_bass_kernel_spmd(
        nc, in_maps, core_ids=list(range(8)), **(_run_kwargs or {})
    )
    out = np.zeros((B, T, D), dtype=np.float32)
    for c in range(8):
        out[c // 4] += res.results[c]["O"].astype(np.float32)
    out += bo
    if _run_kwargs:
        kernel.last_result = res
    return out
